# revision 61
# baseline (speedup 1.0000x reference)
"""GAT SubNet Trainium2 kernel (Bass/Tile, SPMD over 8 NeuronCores).

Reference computation (per batch element b):
  L1 (8 heads): Wh = x@W_h;  e = leaky(s_m + t_k, 0.2) masked by graph;
                att = softmax_k(e);  h = leaky(concat_h(att@Wh))
  L2 (1 head):  same GAT layer with Wo on h;  out = leaky(...)

Sharding: data-parallel over B=16 -> 2 batch elements per core; graph and
weights replicated.

Layout strategy ("layout T"): attention tiles are stored [k on partitions,
m on free] so the att@Wh matmul contracts k on partitions directly (no
per-tile transposes) and the softmax denominator rides the matmul as a
ones column. The additive mask (-30000 for absent edges) folds into one
fused DVE scalar_tensor_tensor op; leaky is a second fused DVE op;
exp runs on ScalarE. Elementwise work in fp16, matmuls fp16 x fp16 -> f32
PSUM.
"""
import numpy as np
from contextlib import ExitStack

import concourse.bass as bass
import concourse.tile as tile
from concourse import mybir
from concourse.masks import make_identity

P = 128
B_FULL, NCORES = 16, 8
BL = B_FULL // NCORES          # 2 batch elements per core
N, C, H, D, O = 1024, 12, 8, 64, 64
KC = N // P                    # 8 chunks of neighbors/rows
F = H * D                      # 512 concat features
FC = F // P                    # 4 feature chunks
NEG = -30000.0                 # additive mask value (safe in fp16)
ALPHA = 0.2

XBAR_MASK = False               # mask transpose via DMA xbar (else PE)
K_ACT_SET = frozenset(range(0, 16, 2))   # L1 blocks whose leaky runs on ACT
EXP_SPLIT = 1                            # how many ACT ops per block's exp
E_BUFS = 2
ET_BUFS = 2
SBC_SWAP = False
FINE_LAST = True
FINE_FIRST = False
FINE_L2N = 1
PS_H_BUFS = 2
PS_ATT_BUFS = 2
K_TSPOOL_SET = frozenset()
K_LPOOL_SET = frozenset()
TBIAS_FOLD = False
SPLIT_LEAKY = False
POOL_MODE = "queue"
NORM_FUSED = False              # fold norm scale into Prelu (ACT) vs DVE mul
XBAR_H = False                  # h transpose via DMA xbar (else PE)

f16 = mybir.dt.float16
f32 = mybir.dt.float32
i32 = mybir.dt.int32
Act = mybir.ActivationFunctionType
Alu = mybir.AluOpType


def _split_excess_waits(nc, max_waits=1):
    """This walrus build accepts only one sem-wait per instruction; move
    overflow waits onto preceding no-ops (streams are in-order, so this is
    semantics-preserving)."""
    n = 0
    for f in nc.m.functions:
        for bb in f.blocks:
            new_list = []
            for inst in bb.instructions:
                si = inst.sync_info
                if si is not None and si.on_wait and len(si.on_wait) > max_waits:
                    waits = list(si.on_wait)
                    overflow, keep = waits[:-max_waits], waits[-max_waits:]
                    ci = 0
                    while overflow:
                        chunk, overflow = overflow[:max_waits], overflow[max_waits:]
                        new_list.append(mybir.InstNoOp(
                            name=f"{inst.name}-waitsplit-{ci}",
                            engine=inst.engine,
                            bass_nofuse=True,
                            sync_info=mybir.SyncInfo(on_wait=chunk, on_update=[]),
                        ))
                        ci += 1
                        n += 1
                    si.on_wait = keep
                new_list.append(inst)
            bb.instructions[:] = new_list
    return n


def build_nc():
    nc = bass.Bass()
    x_d = nc.dram_tensor("x", [BL, N, C], f32, kind="ExternalInput")
    g_d = nc.dram_tensor("graph", [N, N], i32, kind="ExternalInput")
    W_d = nc.dram_tensor("W", [H, C, D], f32, kind="ExternalInput")
    asrc_d = nc.dram_tensor("a_src", [H, D], f32, kind="ExternalInput")
    adst_d = nc.dram_tensor("a_dst", [H, D], f32, kind="ExternalInput")
    Wo_d = nc.dram_tensor("Wo", [F, O], f32, kind="ExternalInput")
    aosrc_d = nc.dram_tensor("ao_src", [O], f32, kind="ExternalInput")
    aodst_d = nc.dram_tensor("ao_dst", [O], f32, kind="ExternalInput")
    out_d = nc.dram_tensor("out", [BL, N, O], f32, kind="ExternalOutput")

    with tile.TileContext(nc, pool_alloc_mode=POOL_MODE) as tc, ExitStack() as ctx:
        consts = ctx.enter_context(tc.tile_pool(name="consts", bufs=1))

        ident = consts.tile([P, P], f16)
        make_identity(nc, ident)

        # persistent tensors (nmask split per k-chunk for fine-grained deps)
        nmask = [consts.tile([P, N], f16, name=f"nmask{kt}") for kt in range(KC)]
        xT = consts.tile([C, BL, N], f16)         # [c, b, n]
        Wf = consts.tile([C, H, D], f16)          # [c, h, d]
        rhs1 = consts.tile([C, H, D + 1], f16)    # [c, h, (W_h | w_t)]
        ws_rep = consts.tile([C, H, P], f16)      # W@a_src replicated over cols
        rhs2 = consts.tile([P, FC, O + 2], f16)   # [f_p, fc, (Wo | w2s | w2t)]
        w2s_rep = consts.tile([P, FC, P], f16)    # Wo@ao_src replicated over cols

        # ---------------- prep phase ----------------
        with ExitStack() as pctx:
            pst = pctx.enter_context(tc.tile_pool(name="pst", bufs=3, space="PSUM"))
            stg = pctx.enter_context(tc.tile_pool(name="stg", bufs=3))

            # mask: load graph rows, affine to {0,-30000}, DMA xbar transpose
            ng = [stg.tile([P, N], f16, tag=f"ng{mt}", name=f"ng{mt}")
                  for mt in range(KC)]
            for mt in range(KC):
                g_sb = stg.tile([P, N], i32, tag="g")
                geng = nc.scalar if mt % 2 == 0 else nc.sync
                geng.dma_start(out=g_sb[:], in_=g_d[mt * P:(mt + 1) * P, :])
                # (g - 1) * 30000 -> {0 -> -30000, 1 -> 0}
                nc.gpsimd.tensor_scalar(
                    out=ng[mt][:], in0=g_sb[:], scalar1=1.0, scalar2=-NEG,
                    op0=Alu.subtract, op1=Alu.mult)
            for kt in range(KC):
                eng = nc.sync if kt % 2 == 0 else nc.scalar
                for mt in range(KC):
                    if XBAR_MASK:
                        eng.dma_start_transpose(
                            out=nmask[kt][:, mt * P:(mt + 1) * P],
                            in_=ng[mt][:, kt * P:(kt + 1) * P])
                    else:
                        ptr = pst.tile([P, P], f16, tag="tr")
                        nc.tensor.transpose(
                            ptr[:], ng[mt][:, kt * P:(kt + 1) * P], ident[:])
                        nc.any.tensor_copy(
                            out=nmask[kt][:, mt * P:(mt + 1) * P], in_=ptr[:])

            # x -> xT (cast fp16 + PE transpose per [128,12] block)
            for b in range(BL):
                x_sb = stg.tile([P, KC, C], f32, tag="x")
                nc.sync.dma_start(
                    out=x_sb[:],
                    in_=x_d[b].rearrange("(kc p) c -> p kc c", p=P))
                x16 = stg.tile([P, KC, C], f16, tag="x16")
                nc.vector.tensor_copy(out=x16[:], in_=x_sb[:])
                for kc in range(KC):
                    ptr = pst.tile([C, P], f16, tag="tr")
                    nc.tensor.transpose(ptr[:], x16[:, kc, :], ident[:])
                    nc.any.tensor_copy(
                        out=xT[:, b, kc * P:(kc + 1) * P], in_=ptr[:])

            # W: load, cast, fold attention vectors
            W_sb = stg.tile([C, H, D], f32, tag="W")
            for h in range(H):
                nc.sync.dma_start(out=W_sb[:, h, :], in_=W_d[h])
            nc.vector.tensor_copy(out=Wf[:], in_=W_sb[:])
            av = stg.tile([D, 2 * H], f32, tag="av")   # a_src | a_dst columns
            for h in range(H):
                nc.sync.dma_start(out=av[:, h:h + 1], in_=asrc_d[h, :, None])
                nc.sync.dma_start(out=av[:, H + h:H + h + 1], in_=adst_d[h, :, None])
            av16 = stg.tile([D, 2 * H], f16, tag="av16")
            nc.vector.tensor_copy(out=av16[:], in_=av[:])
            for h in range(H):
                # W_h^T via PE, then w_s = W_h @ a_src, w_t = W_h @ a_dst
                ptr = pst.tile([D, C], f16, tag="tr")
                nc.tensor.transpose(ptr[:], Wf[:, h, :], ident[:C, :C])
                WhT = stg.tile([D, C], f16, tag="wht")
                nc.any.tensor_copy(out=WhT[:], in_=ptr[:])
                pws = pst.tile([C, 2], f32, tag="pws")
                # (tag "pws" shared with the Wo fold below)
                nc.tensor.matmul(pws[:, 0:1], WhT[:], av16[:, h:h + 1],
                                 start=True, stop=True)
                nc.tensor.matmul(pws[:, 1:2], WhT[:], av16[:, H + h:H + h + 1],
                                 start=True, stop=True)
                wsc = stg.tile([C, 1], f16, tag="wsc")
                nc.vector.tensor_copy(out=wsc[:], in_=pws[:, 0:1])
                nc.vector.tensor_copy(out=rhs1[:, h, D:D + 1], in_=pws[:, 1:2])
                nc.vector.tensor_copy(out=rhs1[:, h, 0:D], in_=Wf[:, h, :])
                nc.vector.tensor_copy(
                    out=ws_rep[:, h, :],
                    in_=wsc[:, 0:1].to_broadcast([C, P]))

            # Wo: load, cast, fold ao vectors
            Wo_sb = stg.tile([P, FC, O], f32, tag="Wo")
            nc.sync.dma_start(
                out=Wo_sb[:], in_=Wo_d.rearrange("(fc p) o -> p fc o", p=P))
            Wo16 = stg.tile([P, FC, O], f16, tag="Wo16")
            nc.vector.tensor_copy(out=Wo16[:], in_=Wo_sb[:])
            nc.vector.tensor_copy(out=rhs2[:, :, 0:O], in_=Wo16[:])
            ao = stg.tile([O, 2], f32, tag="ao")
            nc.sync.dma_start(out=ao[:, 0:1], in_=aosrc_d[:, None])
            nc.sync.dma_start(out=ao[:, 1:2], in_=aodst_d[:, None])
            ao16 = stg.tile([O, 2], f16, tag="ao16")
            nc.vector.tensor_copy(out=ao16[:], in_=ao[:])
            for fc in range(FC):
                ptr = pst.tile([O, P], f16, tag="tr")
                nc.tensor.transpose(ptr[:], Wo16[:, fc, :], ident[:])
                WoT = stg.tile([O, P], f16, tag="wot")
                nc.any.tensor_copy(out=WoT[:], in_=ptr[:])
                pws = pst.tile([P, 2], f32, tag="pws")
                nc.tensor.matmul(pws[:, 0:1], WoT[:], ao16[:, 0:1],
                                 start=True, stop=True)
                nc.tensor.matmul(pws[:, 1:2], WoT[:], ao16[:, 1:2],
                                 start=True, stop=True)
                nc.vector.tensor_copy(out=rhs2[:, fc, O:O + 2], in_=pws[:])
                nc.vector.tensor_copy(
                    out=w2s_rep[:, fc, :],
                    in_=rhs2[:, fc, O:O + 1].to_broadcast([P, P]))

        # ---------------- main pools ----------------
        ps_sbc = ctx.enter_context(tc.tile_pool(
            name="ps_sbc", bufs=2 if XBAR_H else 1, space="PSUM"))
        ps_h = None if XBAR_H else ctx.enter_context(
            tc.tile_pool(name="ps_h", bufs=PS_H_BUFS, space="PSUM"))
        ps_wh = ctx.enter_context(tc.tile_pool(name="ps_wh", bufs=2, space="PSUM"))
        ps_att = ctx.enter_context(tc.tile_pool(name="ps_att", bufs=PS_ATT_BUFS, space="PSUM"))

        sb_e = ctx.enter_context(tc.tile_pool(name="sb_e", bufs=E_BUFS))
        sb_et = ctx.enter_context(tc.tile_pool(name="sb_et", bufs=ET_BUFS))
        sb_wh = ctx.enter_context(tc.tile_pool(name="sb_wh", bufs=2))
        sb_misc = ctx.enter_context(tc.tile_pool(name="sb_misc", bufs=2))
        sb_h = ctx.enter_context(tc.tile_pool(name="sb_h", bufs=2))
        sb_out = ctx.enter_context(tc.tile_pool(name="sb_out", bufs=4))

        def attention(b, sbc_lhs_list, t_cols, whones, leaky_on_act, sbc_on_act=True,
                      fine=False, ts_on_pool=False, leaky_on_pool=False):
            """Shared L1/L2 attention block.
            sbc_lhs_list: list of (lhsT_ap, rhs_ap) accumulated for s_bcast.
            t_cols: AP [P, KC] f32 per-partition t columns.
            whones: AP [P, KC, O+1] fp16 (values | ones).
            leaky_on_act: engine-balance knob (leaky via ACT Prelu vs DVE).
            Returns list of psum tiles [(ps, mt_base)] with [P, 4, O+1] each.
            """
            # s broadcast: all partitions hold s[m]
            psb = ps_sbc.tile([P, N], f32, tag="sbc")
            nk = len(sbc_lhs_list)
            for i, (lh, rh) in enumerate(sbc_lhs_list):
                for half in range(2):
                    nc.tensor.matmul(
                        psb[:, half * 512:(half + 1) * 512],
                        lh, rh[:, half * 512:(half + 1) * 512],
                        start=(i == 0), stop=(i == nk - 1))
            sbc = sb_misc.tile([P, N], f16, tag="sbc_sb")
            if sbc_on_act:
                nc.scalar.copy(out=sbc[:], in_=psb[:])
            else:
                nc.vector.tensor_copy(out=sbc[:], in_=psb[:])

            # E tiles: e = (mask + t_k) + s_m ; leaky ; exp
            if fine:
                # per-kc tiles: matmuls unblock per chunk (shorter drain for
                # the last block); slightly more per-op overhead
                Ek = [sb_e.tile([P, N], f16, tag=f"Ek{kc}", name=f"Ek{kc}")
                      for kc in range(KC)]
                for kc in range(KC):
                    et = sb_et.tile([P, N], f16, tag="Etf")
                    nc.vector.tensor_scalar_add(
                        out=et[:], in0=nmask[kc][:],
                        scalar1=t_cols[:, kc:kc + 1])
                    nc.vector.tensor_tensor(out=et[:], in0=et[:], in1=sbc[:],
                                            op=Alu.add)
                    nc.scalar.activation(out=Ek[kc][:], in_=et[:],
                                         func=Act.Prelu, alpha=ALPHA)
                    nc.scalar.activation(out=Ek[kc][:], in_=Ek[kc][:],
                                         func=Act.Exp)
                eview = lambda kc: Ek[kc][:]
            else:
                Et = sb_et.tile([P, KC, N], f16, tag="Etmp")
                E = sb_e.tile([P, KC, N], f16, tag="E")
                if leaky_on_act and TBIAS_FOLD:
                    # t folds into per-kc Prelu bias; DVE does only the
                    # nmask+s_bcast adds (per-kc TT at 2x)
                    for kc in range(KC):
                        nc.vector.tensor_tensor(
                            out=Et[:, kc, :], in0=nmask[kc][:], in1=sbc[:],
                            op=Alu.add)
                        nc.scalar.activation(
                            out=E[:, kc, :], in_=Et[:, kc, :], func=Act.Prelu,
                            alpha=ALPHA, bias=t_cols[:, kc:kc + 1])
                    ks = KC // EXP_SPLIT
                    for i in range(EXP_SPLIT):
                        nc.scalar.activation(out=E[:, i * ks:(i + 1) * ks, :],
                                             in_=E[:, i * ks:(i + 1) * ks, :],
                                             func=Act.Exp)
                    eview = lambda kc: E[:, kc, :]
                    outs = []
                    for half in range(2):
                        ps = ps_att.tile([P, 4, O + 1], f32, tag="att")
                        for mi in range(4):
                            mt = half * 4 + mi
                            for kc in range(KC):
                                nc.tensor.matmul(
                                    ps[:, mi, :],
                                    eview(kc)[:, mt * P:(mt + 1) * P],
                                    whones[:, kc, :],
                                    start=(kc == 0), stop=(kc == KC - 1))
                        outs.append((ps, half * 4))
                    return outs
                ts_eng = nc.gpsimd if ts_on_pool else nc.vector
                for kc in range(KC):   # 4x-mode tensor_scalar (DVE) / Pool
                    ts_eng.tensor_scalar_add(
                        out=Et[:, kc, :], in0=nmask[kc][:],
                        scalar1=t_cols[:, kc:kc + 1])
                nc.vector.tensor_tensor(   # 2x-mode, s_bcast broadcast over kc
                    out=Et[:], in0=Et[:],
                    in1=sbc[:, None, :].to_broadcast([P, KC, N]), op=Alu.add)
                if leaky_on_act:
                    nc.scalar.activation(out=E[:], in_=Et[:],
                                         func=Act.Prelu, alpha=ALPHA)
                elif leaky_on_pool:
                    nc.gpsimd.tensor_scalar_mul(out=E[:], in0=Et[:], scalar1=ALPHA)
                    nc.gpsimd.tensor_max(out=E[:], in0=E[:], in1=Et[:])
                else:
                    nc.vector.tensor_scalar_mul(out=E[:], in0=Et[:], scalar1=ALPHA)
                    nc.vector.tensor_tensor(out=E[:], in0=E[:], in1=Et[:],
                                            op=Alu.max)
                ks = KC // EXP_SPLIT
                for i in range(EXP_SPLIT):
                    nc.scalar.activation(out=E[:, i * ks:(i + 1) * ks, :],
                                         in_=E[:, i * ks:(i + 1) * ks, :],
                                         func=Act.Exp)
                eview = lambda kc: E[:, kc, :]

            # att @ [values | ones] accumulated over kc
            outs = []
            for half in range(2):
                ps = ps_att.tile([P, 4, O + 1], f32, tag="att")
                for mi in range(4):
                    mt = half * 4 + mi
                    for kc in range(KC):
                        nc.tensor.matmul(
                            ps[:, mi, :],
                            eview(kc)[:, mt * P:(mt + 1) * P],
                            whones[:, kc, :],
                            start=(kc == 0), stop=(kc == KC - 1))
                outs.append((ps, half * 4))
            return outs

        for b in range(BL):
            # per-fc tiles: hT transposes for chunk fc unblock after its 2 heads
            h_sbs = [sb_h.tile([P, KC, P], f16, tag=f"h{fc}", name=f"h{fc}_{b}")
                     for fc in range(FC)]

            # ---------- layer 1, 8 heads (software-pipelined) ----------
            def l1_prep(h):
                # Wh and t columns in one matmul: rhs = [W_h | w_t]
                whones = sb_wh.tile([P, KC, D + 1], f16, tag="whones",
                                    name=f"who_{b}_{h}")
                tcol = sb_misc.tile([P, KC], f32, tag="tcol",
                                    name=f"tc_{b}_{h}")
                for half in range(2):
                    pwh = ps_wh.tile([P, 4, D + 1], f32, tag="wh",
                                     name=f"pwh_{b}_{h}_{half}")
                    for ki in range(4):
                        kc = half * 4 + ki
                        nc.tensor.matmul(
                            pwh[:, ki, :],
                            xT[:, b, kc * P:(kc + 1) * P], rhs1[:, h, :],
                            start=True, stop=True)
                    nc.vector.tensor_copy(
                        out=whones[:, half * 4:(half + 1) * 4, 0:D],
                        in_=pwh[:, :, 0:D])
                    nc.vector.tensor_copy(
                        out=tcol[:, half * 4:(half + 1) * 4],
                        in_=pwh[:, :, D:D + 1].rearrange("p a b -> p (a b)"))
                nc.vector.memset(whones[:, :, D:D + 1], 1.0)
                # s broadcast + e = (mask + t) + s  (DVE stages)
                psb = ps_sbc.tile([P, N], f32, tag="sbc", name=f"psb_{b}_{h}")
                for half in range(2):
                    nc.tensor.matmul(
                        psb[:, half * 512:(half + 1) * 512],
                        ws_rep[:, h, :],
                        xT[:, b, half * 512:(half + 1) * 512],
                        start=True, stop=True)
                sbc = sb_misc.tile([P, N], f16, tag="sbc_sb",
                                   name=f"sbc_{b}_{h}")
                nc.scalar.copy(out=sbc[:], in_=psb[:])
                Et = sb_et.tile([P, KC, N], f16, tag="Etmp", name=f"et_{b}_{h}")
                for kc in range(KC):
                    nc.vector.tensor_scalar_add(
                        out=Et[:, kc, :], in0=nmask[kc][:],
                        scalar1=tcol[:, kc:kc + 1])
                nc.vector.tensor_tensor(
                    out=Et[:], in0=Et[:],
                    in1=sbc[:, None, :].to_broadcast([P, KC, N]), op=Alu.add)
                return whones, Et

            def l1_fire(h, whones, Et, loa):
                E = sb_e.tile([P, KC, N], f16, tag="E", name=f"e_{b}_{h}")
                if SPLIT_LEAKY:
                    # co-run leaky: ACT does chunks 0..3, DVE does 4..7
                    nc.scalar.activation(out=E[:, 0:4, :], in_=Et[:, 0:4, :],
                                         func=Act.Prelu, alpha=ALPHA)
                    nc.vector.tensor_scalar_mul(out=E[:, 4:KC, :],
                                                in0=Et[:, 4:KC, :],
                                                scalar1=ALPHA)
                    nc.vector.tensor_tensor(out=E[:, 4:KC, :],
                                            in0=E[:, 4:KC, :],
                                            in1=Et[:, 4:KC, :], op=Alu.max)
                elif loa:
                    nc.scalar.activation(out=E[:], in_=Et[:],
                                         func=Act.Prelu, alpha=ALPHA)
                else:
                    nc.vector.tensor_scalar_mul(out=E[:], in0=Et[:],
                                                scalar1=ALPHA)
                    nc.vector.tensor_tensor(out=E[:], in0=E[:], in1=Et[:],
                                            op=Alu.max)
                nc.scalar.activation(out=E[:], in_=E[:], func=Act.Exp)
                outs = []
                for half in range(2):
                    ps = ps_att.tile([P, 4, D + 1], f32, tag="att",
                                     name=f"ps_{b}_{h}_{half}")
                    for mi in range(4):
                        mt = half * 4 + mi
                        for kc in range(KC):
                            nc.tensor.matmul(
                                ps[:, mi, :],
                                E[:, kc, mt * P:(mt + 1) * P],
                                whones[:, kc, :],
                                start=(kc == 0), stop=(kc == KC - 1))
                    outs.append((ps, half * 4))
                return outs

            state = l1_prep(0)
            for h in range(H):
                nstate = l1_prep(h + 1) if h + 1 < H else None
                loa = (b * H + h) in K_ACT_SET
                att_out = l1_fire(h, state[0], state[1], loa)
                state = nstate

                # normalize + leaky -> h slice (f = h*64 .. h*64+63)
                for ps, mt0 in att_out:
                    z = sb_misc.tile([P, 4], f32, tag="z")
                    nc.vector.reciprocal(out=z[:], in_=ps[:, :, D:D + 1].rearrange("p a b -> p (a b)"))
                    if NORM_FUSED:
                        for mi in range(4):
                            nc.scalar.activation(
                                out=h_sbs[h // 2][:, mt0 + mi,
                                                  (h % 2) * D:(h % 2 + 1) * D],
                                in_=ps[:, mi, 0:D], func=Act.Prelu, alpha=ALPHA,
                                scale=z[:, mi:mi + 1])
                    else:
                        L = sb_misc.tile([P, 4, D], f32, tag="L")
                        nc.scalar.activation(out=L[:], in_=ps[:, :, 0:D],
                                             func=Act.Prelu, alpha=ALPHA)
                        for mi in range(4):
                            nc.vector.tensor_scalar_mul(
                                out=h_sbs[h // 2][:, mt0 + mi,
                                                  (h % 2) * D:(h % 2 + 1) * D],
                                in0=L[:, mi, :], scalar1=z[:, mi:mi + 1])

            # ---------- h^T via DMA xbar transpose ----------
            # per-fc tiles so layer-2 consumers unblock as each fc finishes
            hT = [sb_h.tile([P, N], f16, tag=f"hT{fc}", name=f"hT{fc}")
                  for fc in range(FC)]
            for fc in range(FC):
                eng = nc.sync if fc % 2 == 0 else nc.scalar
                for mt in range(KC):
                    if XBAR_H:
                        eng.dma_start_transpose(
                            out=hT[fc][:, mt * P:(mt + 1) * P],
                            in_=h_sbs[fc][:, mt, :])
                    else:
                        ptr = ps_h.tile([P, P], f16, tag="htr")
                        nc.tensor.transpose(
                            ptr[:], h_sbs[fc][:, mt, :], ident[:])
                        nc.any.tensor_copy(
                            out=hT[fc][:, mt * P:(mt + 1) * P], in_=ptr[:])

            # ---------- layer 2 ----------
            # Wh2 (+ s2,t2 cols): [n_p, nt, (O | s2 | t2)]
            w2ones = sb_wh.tile([P, KC, O + 1], f16, tag="w2ones")
            t2 = sb_misc.tile([P, KC], f32, tag="t2")
            for half in range(2):
                pw2 = ps_wh.tile([P, 4, O + 2], f32, tag="wh")
                for ni in range(4):
                    nt = half * 4 + ni
                    for fc in range(FC):
                        nc.tensor.matmul(
                            pw2[:, ni, :],
                            hT[fc][:, nt * P:(nt + 1) * P], rhs2[:, fc, :],
                            start=(fc == 0), stop=(fc == FC - 1))
                nc.vector.tensor_copy(
                    out=w2ones[:, half * 4:(half + 1) * 4, 0:O],
                    in_=pw2[:, :, 0:O])
                nc.vector.tensor_copy(
                    out=t2[:, half * 4:(half + 1) * 4],
                    in_=pw2[:, :, O + 1:O + 2].rearrange("p a b -> p (a b)"))
            nc.vector.memset(w2ones[:, :, O:O + 1], 1.0)

            att2_out = attention(
                b,
                [(w2s_rep[:, fc, :], hT[fc][:]) for fc in range(FC)],
                t2[:],
                w2ones[:],
                leaky_on_act=True,
                fine=(FINE_LAST and b >= BL - FINE_L2N))

            for ps, mt0 in att2_out:
                z = sb_misc.tile([P, 4], f32, tag="z2")
                nc.vector.reciprocal(out=z[:], in_=ps[:, :, O:O + 1].rearrange("p a b -> p (a b)"))
                if NORM_FUSED:
                    for mi in range(4):
                        ot = sb_out.tile([P, O], f32, tag="o")
                        nc.scalar.activation(
                            out=ot[:], in_=ps[:, mi, 0:O], func=Act.Prelu,
                            alpha=ALPHA, scale=z[:, mi:mi + 1])
                        mt = mt0 + mi
                        nc.sync.dma_start(
                            out=out_d[b, mt * P:(mt + 1) * P, :], in_=ot[:])
                else:
                    L = sb_misc.tile([P, 4, O], f32, tag="L2")
                    nc.scalar.activation(out=L[:], in_=ps[:, :, 0:O],
                                         func=Act.Prelu, alpha=ALPHA)
                    for mi in range(4):
                        ot = sb_out.tile([P, O], f32, tag="o")
                        nc.vector.tensor_scalar_mul(
                            out=ot[:], in0=L[:, mi, :], scalar1=z[:, mi:mi + 1])
                        mt = mt0 + mi
                        nc.sync.dma_start(
                            out=out_d[b, mt * P:(mt + 1) * P, :], in_=ot[:])

    _split_excess_waits(nc, 1)
    return nc


_NC_CACHE = None


def _get_nc():
    global _NC_CACHE
    if _NC_CACHE is None:
        _NC_CACHE = build_nc()
    return _NC_CACHE


def kernel(x, graph, W, a_src, a_dst, Wo, ao_src, ao_dst):
    from concourse.bass_utils import run_bass_kernel_spmd
    x = np.asarray(x, dtype=np.float32)
    graph = np.ascontiguousarray(np.asarray(graph, dtype=np.int32))
    W = np.asarray(W, dtype=np.float32)
    a_src = np.asarray(a_src, dtype=np.float32)
    a_dst = np.asarray(a_dst, dtype=np.float32)
    Wo = np.asarray(Wo, dtype=np.float32)
    ao_src = np.asarray(ao_src, dtype=np.float32)
    ao_dst = np.asarray(ao_dst, dtype=np.float32)

    nc = _get_nc()
    in_maps = []
    for c in range(NCORES):
        in_maps.append({
            "x": np.ascontiguousarray(x[c * BL:(c + 1) * BL]),
            "graph": graph,
            "W": W, "a_src": a_src, "a_dst": a_dst,
            "Wo": Wo, "ao_src": ao_src, "ao_dst": ao_dst,
        })
    res = run_bass_kernel_spmd(nc, in_maps, list(range(NCORES)))
    return np.concatenate([res.results[c]["out"] for c in range(NCORES)], axis=0)
